# revision 16
# baseline (speedup 1.0000x reference)
"""De-stationary causal attention (B=2, L=S=2048, H=8, E=64) on 8 TRN2 cores.

Sharding: the 16 (batch, head) pairs are distributed 2-per-core (cores 0-3
get batch 0, heads 0..7; cores 4-7 get batch 1). Each core runs the same
Bass program on its two pairs.

Math: logits = (Q K^T) * (tau/sqrt(E)) + delta/sqrt(E), causal softmax, A V.
Host-side folds: Q is pre-scaled by tau/sqrt(E); exp(delta/sqrt(E)) is folded
into V (and into the appended denominator column), because
softmax(x + d)_s = exp(x_s) e^{d_s} / sum_j exp(x_j) e^{d_j}.
So the device only computes exp(q'k) with no bias, letting one ACT call span
a whole 4-bank PSUM group.

Device structure per (b,h) pair, scores kept TRANSPOSED (s on partitions):
  bank-major over 4 output l-blocks of 512; for each bank, groups of 4
  s-tiles: ST[s,l] row-packed on the PE (two k=64 matmuls on partition halves
  run concurrently), one exp over the [128,2048] group, causal mask on diag
  blocks, then AV row-packed into two accumulators (k split 64+64), merged on
  the DVE, PE-transposed, normalized by the denominator column, stored.
"""

import copy
import sys

import numpy as np

try:
    import concourse.bass as bass
except ImportError:  # pragma: no cover
    sys.path.insert(0, "/opt/trn_rl_repo")
    import concourse.bass as bass

import concourse.mybir as mybir
import concourse.tile as tile
from concourse.bass_utils import run_bass_kernel_spmd
from concourse.vector_clock import ScopedClock

B, L, H, E = 2, 2048, 8, 64
N_CORES = 8
PAIRS_PER_CORE = 2
SCALE = 1.0 / np.sqrt(np.float32(E))  # 0.125

f32 = mybir.dt.float32
f32r = mybir.dt.float32r
bf16 = mybir.dt.bfloat16

# ---------------------------------------------------------------------------
# Walrus in this toolchain rejects >1 sync-wait per instruction. Split extra
# waits onto NoOps committed just before the instruction on the same engine.
# ---------------------------------------------------------------------------
_NOP_TEMPLATE = {}


def _make_nop(engine, name):
    if engine not in _NOP_TEMPLATE:
        tmp = bass.Bass()
        _NOP_TEMPLATE[engine] = tmp.engines[engine].nop(nofuse=True).ins
    nop = copy.copy(_NOP_TEMPLATE[engine])
    nop.name = name
    nop.engine = engine
    nop.sync_info = None
    return nop


class SplitWaitTileContext(tile.TileContext):
    _ws_counter = 0

    def _split_waits(self, inst):
        si = inst.sync_info
        if si is None or not si.on_wait or len(si.on_wait) <= 1:
            return []
        if inst.engine == mybir.EngineType.Unassigned:
            return []
        waits = list(si.on_wait)
        inst.sync_info = mybir.SyncInfo(
            on_wait=[waits[0]], on_update=list(si.on_update or [])
        )
        nops = []
        for w in waits[1:]:
            SplitWaitTileContext._ws_counter += 1
            nop = _make_nop(inst.engine, f"I-ws{SplitWaitTileContext._ws_counter}")
            nop.sync_info = mybir.SyncInfo(on_wait=[w], on_update=[])
            nops.append(nop)
        return nops

    def _commit_instruction(self, inst, lazy_reg_writes=True):
        for nop in self._split_waits(inst):
            self._add_instruction(nop)
        super()._commit_instruction(inst, lazy_reg_writes)

    def _drain_and_barrier(self, tick_clock, wait_clock):
        nc = self.nc
        probe = nc.sync.nop(nofuse=True)
        wait_clock.add_sem_waits(
            probe.ins, ScopedClock({None: tick_clock.global_clock})
        )
        waits = list(probe.ins.sync_info.on_wait or []) if probe.ins.sync_info else []
        if len(waits) > 1:
            probe.ins.sync_info.on_wait = [waits[0]]
            handles = {h.num: h for h in self.sems.allocated().values()}
            for w in waits[1:]:
                nop = nc.sync.nop(nofuse=True)
                nop.wait_op(handles[w.id], w.wait_value, "sem-ge")
        nc.sync.drain()

        nc.all_engine_barrier()
        assert self.sems is not None
        popped = nc._tile_sem_poison_stack.pop()
        assert popped is self._sem_poison
        nc.clear_and_free_semaphores(list(self.sems.allocated().values()))
        nc.all_engine_barrier()


# ---------------------------------------------------------------------------
# Program builder (bank-major, fully row-packed)
# ---------------------------------------------------------------------------

def build_program(st_dtype=f32r, av_dtype=f32r):
    nc = bass.Bass()
    Exp = mybir.ActivationFunctionType.Exp

    VW = E + 2  # v row: 64 values + denominator col + pad
    qt = nc.declare_dram_parameter("qt", [PAIRS_PER_CORE, E, L], st_dtype, isOutput=False)
    kt = nc.declare_dram_parameter("kt", [PAIRS_PER_CORE, E, L], st_dtype, isOutput=False)
    vv = nc.declare_dram_parameter("vv", [PAIRS_PER_CORE, L, VW], av_dtype, isOutput=False)
    mask = nc.declare_dram_parameter("mask", [128, 128], av_dtype, isOutput=False)
    ident = nc.declare_dram_parameter("ident", [128, 128], f32, isOutput=False)
    oo = nc.declare_dram_parameter("oo", [PAIRS_PER_CORE, L, E], f32, isOutput=True)

    NT = L // 128  # 16 s-tiles / l-tiles
    NB = L // 512  # 4 OT banks

    with SplitWaitTileContext(nc) as tc:
        with (
            tc.tile_pool(name="const", bufs=1) as constp,
            tc.tile_pool(name="qk", bufs=2) as qkp,
            tc.tile_pool(name="vp", bufs=2) as vp,
            tc.tile_pool(name="ap", bufs=3) as ap_pool,
            tc.tile_pool(name="ep", bufs=2) as ep,
            tc.tile_pool(name="outp", bufs=2) as outp,
            tc.tile_pool(name="st", bufs=1, space="PSUM") as stp,
            tc.tile_pool(name="otp", bufs=2, space="PSUM") as otp,
            tc.tile_pool(name="ottp", bufs=2, space="PSUM") as ottp,
        ):
            mask_sb = constp.tile([128, 128], av_dtype, tag="mask")
            nc.sync.dma_start(out=mask_sb, in_=mask[:])
            ident_sb = constp.tile([128, 128], f32, tag="ident")
            nc.sync.dma_start(out=ident_sb, in_=ident[:])

            for pair in range(PAIRS_PER_CORE):
                # Q^T/K^T duplicated on partitions 0-63 and 64-127 so the
                # k=64 score matmuls can row-pack two s-tiles concurrently
                qt_sb = qkp.tile([2 * E, L], st_dtype, tag="qt")
                kt_sb = qkp.tile([2 * E, L], st_dtype, tag="kt")
                # V slab [128, 16, 66]; col 64 carries exp(delta') for the
                # softmax denominator (host-folded), col 65 is padding
                v_sb = vp.tile([128, NT, VW], av_dtype, tag="v")
                vv_r = vv[pair].rearrange("(t p) e -> p t e", p=128)
                # first 512 cols small + fast (feeds group (0,0)), remainder
                # as one big transfer per half; dup/V issued on the scalar
                # engine's HWDGE queue to unload the sync sequencer
                c0 = slice(0, 512)
                cr = slice(512, L)
                nc.sync.dma_start(out=kt_sb[0:E, c0], in_=kt[pair][:, c0])
                nc.scalar.dma_start(out=kt_sb[E : 2 * E, c0], in_=kt[pair][:, c0])
                nc.sync.dma_start(out=qt_sb[0:E, c0], in_=qt[pair][:, c0])
                nc.scalar.dma_start(out=qt_sb[E : 2 * E, c0], in_=qt[pair][:, c0])
                nc.sync.dma_start(out=v_sb[:, 0:4, :], in_=vv_r[:, 0:4, :])
                nc.sync.dma_start(out=kt_sb[0:E, cr], in_=kt[pair][:, cr])
                nc.scalar.dma_start(out=kt_sb[E : 2 * E, cr], in_=kt[pair][:, cr])
                nc.sync.dma_start(out=qt_sb[0:E, cr], in_=qt[pair][:, cr])
                nc.scalar.dma_start(out=qt_sb[E : 2 * E, cr], in_=qt[pair][:, cr])
                nc.sync.dma_start(out=v_sb[:, 4:NT, :], in_=vv_r[:, 4:NT, :])

                out_sb = outp.tile([128, NT, E], f32, tag="out")

                ot_banks = {}

                def emit_st_group(lj, gi):
                    a_grp = ap_pool.tile(
                        [128, 4 * 512], av_dtype, tag="A", name="A"
                    )
                    for hb in range(2):  # two double-buffered [128,1024] halves
                        st = stp.tile(
                            [128, 1024], f32, tag=f"st{hb}", name="st"
                        )
                        for cc in range(2):
                            c = 2 * hb + cc
                            si = 4 * gi + c
                            off = 128 * c if gi == lj else 0
                            half = (c % 2) * E
                            nc.tensor.matmul(
                                st[:, 512 * cc + off : 512 * (cc + 1)],
                                kt_sb[half : half + E, si * 128 : si * 128 + 128],
                                qt_sb[half : half + E, 512 * lj + off : 512 * lj + 512],
                                start=True,
                                stop=True,
                            )
                        if gi == lj:
                            # diagonal group: exp only the valid suffix of
                            # each 512-chunk (skips unwritten PSUM columns)
                            for cc in range(2):
                                c = 2 * hb + cc
                                off = 128 * c
                                nc.scalar.activation(
                                    out=a_grp[
                                        :, 512 * c + off : 512 * (c + 1)
                                    ],
                                    in_=st[:, 512 * cc + off : 512 * (cc + 1)],
                                    func=Exp,
                                    scale=1.0,
                                )
                        else:
                            nc.scalar.activation(
                                out=a_grp[:, 1024 * hb : 1024 * (hb + 1)],
                                in_=st,
                                func=Exp,
                                scale=1.0,
                            )
                    if gi == lj:
                        for c in range(4):
                            colb = 512 * c + 128 * c
                            nc.vector.tensor_mul(
                                a_grp[:, colb : colb + 128],
                                a_grp[:, colb : colb + 128],
                                mask_sb,
                            )
                    return a_grp

                def emit_av_group(lj, gi, a_grp):
                    ot = ot_banks[lj]
                    for c in range(4):
                        si = 4 * gi + c
                        off = 128 * c if gi == lj else 0
                        nc.tensor.matmul(
                            ot[:, off:512],
                            v_sb[:, si, 0 : E + 1],
                            a_grp[:, 512 * c + off : 512 * (c + 1)],
                            start=(gi == 0 and c == 0),
                            stop=(gi == lj and c == 3),
                        )

                def emit_epilogue(lj):
                    ot = ot_banks.pop(lj)
                    ot_sb = ep.tile([E + 1, 512], f32, tag="ot_sb", name="ot_sb")
                    nc.vector.tensor_copy(ot_sb, ot)
                    for c in range(4):
                        lt = 4 * lj + c
                        ott = ottp.tile([128, 512], f32, tag="ott", name="ott")
                        nc.tensor.transpose(
                            ott[:, 0 : E + 1],
                            ot_sb[:, c * 128 : (c + 1) * 128],
                            ident_sb[0 : E + 1, 0 : E + 1],
                        )
                        recip = ep.tile([128, 1], f32, tag="recip", name="recip")
                        nc.vector.reciprocal(recip, ott[:, E : E + 1])
                        nc.vector.tensor_scalar_mul(
                            out_sb[:, lt, :], ott[:, 0:E], recip
                        )

                # groups: (lj, gi) — bank lj accumulates s-tiles 0..4lj+3 in
                # groups of 4; gi == lj is the diagonal (partial) group.
                # Software-pipelined: PE stays one group ahead of AV.
                groups = [(lj, gi) for lj in range(NB) for gi in range(lj + 1)]
                prev = None
                for lj, gi in groups:
                    if lj not in ot_banks:
                        ot_banks[lj] = otp.tile(
                            [E + 1, 512], f32, tag="ot", name="ot"
                        )
                    a_grp = emit_st_group(lj, gi)
                    if prev is not None:
                        plj, pgi, pa = prev
                        emit_av_group(plj, pgi, pa)
                        if pgi == plj:  # that was the last group of bank plj
                            emit_epilogue(plj)
                    prev = (lj, gi, a_grp)
                plj, pgi, pa = prev
                emit_av_group(plj, pgi, pa)
                emit_epilogue(plj)

                nc.sync.dma_start(
                    out=oo[pair].rearrange("(t p) e -> p t e", p=128),
                    in_=out_sb,
                )

    return nc


# ---------------------------------------------------------------------------
# Host-side sharding / unsharding
# ---------------------------------------------------------------------------

def _in_maps(queries, keys, values, tau, delta, st_dtype=f32r, av_dtype=f32r):
    np_st = mybir.dt.np(st_dtype)
    np_av = mybir.dt.np(av_dtype)
    mask = np.triu(np.ones((128, 128), dtype=np.float32)).astype(np_av)
    ident = np.eye(128, dtype=np.float32)
    maps = []
    for c in range(N_CORES):
        ps = [2 * c, 2 * c + 1]
        b = ps[0] // H
        hs = [p % H for p in ps]
        qscale = np.float32(SCALE * tau[b, 0])
        qt = np.ascontiguousarray(
            np.stack([queries[b, :, h, :].T * qscale for h in hs])
        ).astype(np_st)
        kt = np.ascontiguousarray(
            np.stack([keys[b, :, h, :].T for h in hs])
        ).astype(np_st)
        # V augmented with the delta fold: cols 0..63 = V * exp(delta'),
        # col 64 = exp(delta') (denominator), col 65 pad
        expd = np.exp(SCALE * delta[b]).astype(np.float32)  # [L]
        vv = np.zeros((PAIRS_PER_CORE, L, E + 2), dtype=np.float32)
        for i, h in enumerate(hs):
            vv[i, :, 0:E] = values[b, :, h, :] * expd[:, None]
            vv[i, :, E] = expd
        vv = np.ascontiguousarray(vv).astype(np_av)
        maps.append(
            {"qt": qt, "kt": kt, "vv": vv, "mask": mask, "ident": ident}
        )
    return maps


_CACHED = {}


def run(queries, keys, values, tau, delta, trace=False, st_dtype=f32r,
        av_dtype=f32r):
    key = (str(st_dtype), str(av_dtype))
    if key not in _CACHED:
        _CACHED[key] = build_program(st_dtype, av_dtype)
    nc = _CACHED[key]
    in_maps = _in_maps(
        np.asarray(queries),
        np.asarray(keys),
        np.asarray(values),
        np.asarray(tau),
        np.asarray(delta),
        st_dtype=st_dtype,
        av_dtype=av_dtype,
    )
    res = run_bass_kernel_spmd(
        nc, in_maps, core_ids=list(range(N_CORES)), trace=trace
    )
    out = np.empty((B, L, H, E), dtype=np.float32)
    for c in range(N_CORES):
        o = res.results[c]["oo"]
        for i, p in enumerate([2 * c, 2 * c + 1]):
            out[p // H, :, p % H, :] = o[i]
    return out, res


def kernel(queries, keys, values, tau, delta):
    out, _ = run(queries, keys, values, tau, delta, trace=False)
    return out


# revision 17
# speedup vs baseline: 1.0156x; 1.0156x over previous
"""De-stationary causal attention (B=2, L=S=2048, H=8, E=64) on 8 TRN2 cores.

Sharding: the 16 (batch, head) pairs are distributed 2-per-core (cores 0-3
get batch 0, heads 0..7; cores 4-7 get batch 1). Each core runs the same
Bass program on its two pairs.

Math: logits = (Q K^T) * (tau/sqrt(E)) + delta/sqrt(E), causal softmax, A V.
Host-side folds: Q is pre-scaled by tau/sqrt(E); exp(delta/sqrt(E)) is folded
into V (and into the appended denominator column), because
softmax(x + d)_s = exp(x_s) e^{d_s} / sum_j exp(x_j) e^{d_j}.
So the device only computes exp(q'k) with no bias, letting one ACT call span
a whole 4-bank PSUM group.

Device structure per (b,h) pair, scores kept TRANSPOSED (s on partitions):
  bank-major over 4 output l-blocks of 512; for each bank, groups of 4
  s-tiles: ST[s,l] row-packed on the PE (two k=64 matmuls on partition halves
  run concurrently), one exp over the [128,2048] group, causal mask on diag
  blocks, then AV row-packed into two accumulators (k split 64+64), merged on
  the DVE, PE-transposed, normalized by the denominator column, stored.
"""

import copy
import sys

import numpy as np

try:
    import concourse.bass as bass
except ImportError:  # pragma: no cover
    sys.path.insert(0, "/opt/trn_rl_repo")
    import concourse.bass as bass

import concourse.mybir as mybir
import concourse.tile as tile
from concourse.bass_utils import run_bass_kernel_spmd
from concourse.vector_clock import ScopedClock

B, L, H, E = 2, 2048, 8, 64
N_CORES = 8
PAIRS_PER_CORE = 2
SCALE = 1.0 / np.sqrt(np.float32(E))  # 0.125

f32 = mybir.dt.float32
f32r = mybir.dt.float32r
bf16 = mybir.dt.bfloat16

# ---------------------------------------------------------------------------
# Walrus in this toolchain rejects >1 sync-wait per instruction. Split extra
# waits onto NoOps committed just before the instruction on the same engine.
# ---------------------------------------------------------------------------
_NOP_TEMPLATE = {}


def _make_nop(engine, name):
    if engine not in _NOP_TEMPLATE:
        tmp = bass.Bass()
        _NOP_TEMPLATE[engine] = tmp.engines[engine].nop(nofuse=True).ins
    nop = copy.copy(_NOP_TEMPLATE[engine])
    nop.name = name
    nop.engine = engine
    nop.sync_info = None
    return nop


class SplitWaitTileContext(tile.TileContext):
    _ws_counter = 0

    def _split_waits(self, inst):
        si = inst.sync_info
        if si is None or not si.on_wait or len(si.on_wait) <= 1:
            return []
        if inst.engine == mybir.EngineType.Unassigned:
            return []
        waits = list(si.on_wait)
        inst.sync_info = mybir.SyncInfo(
            on_wait=[waits[0]], on_update=list(si.on_update or [])
        )
        nops = []
        for w in waits[1:]:
            SplitWaitTileContext._ws_counter += 1
            nop = _make_nop(inst.engine, f"I-ws{SplitWaitTileContext._ws_counter}")
            nop.sync_info = mybir.SyncInfo(on_wait=[w], on_update=[])
            nops.append(nop)
        return nops

    def _commit_instruction(self, inst, lazy_reg_writes=True):
        for nop in self._split_waits(inst):
            self._add_instruction(nop)
        super()._commit_instruction(inst, lazy_reg_writes)

    def _drain_and_barrier(self, tick_clock, wait_clock):
        nc = self.nc
        probe = nc.sync.nop(nofuse=True)
        wait_clock.add_sem_waits(
            probe.ins, ScopedClock({None: tick_clock.global_clock})
        )
        waits = list(probe.ins.sync_info.on_wait or []) if probe.ins.sync_info else []
        if len(waits) > 1:
            probe.ins.sync_info.on_wait = [waits[0]]
            handles = {h.num: h for h in self.sems.allocated().values()}
            for w in waits[1:]:
                nop = nc.sync.nop(nofuse=True)
                nop.wait_op(handles[w.id], w.wait_value, "sem-ge")
        nc.sync.drain()

        nc.all_engine_barrier()
        assert self.sems is not None
        popped = nc._tile_sem_poison_stack.pop()
        assert popped is self._sem_poison
        nc.clear_and_free_semaphores(list(self.sems.allocated().values()))
        nc.all_engine_barrier()


# ---------------------------------------------------------------------------
# Program builder (bank-major, fully row-packed)
# ---------------------------------------------------------------------------

def build_program(st_dtype=f32r, av_dtype=f32r):
    nc = bass.Bass()
    Exp = mybir.ActivationFunctionType.Exp

    VW = E + 2  # v row: 64 values + denominator col + pad
    qt = nc.declare_dram_parameter("qt", [PAIRS_PER_CORE, E, L], st_dtype, isOutput=False)
    kt = nc.declare_dram_parameter("kt", [PAIRS_PER_CORE, E, L], st_dtype, isOutput=False)
    vv = nc.declare_dram_parameter("vv", [PAIRS_PER_CORE, L, VW], av_dtype, isOutput=False)
    mask = nc.declare_dram_parameter("mask", [128, 128], av_dtype, isOutput=False)
    ident = nc.declare_dram_parameter("ident", [128, 128], f32, isOutput=False)
    oo = nc.declare_dram_parameter("oo", [PAIRS_PER_CORE, L, E], f32, isOutput=True)

    NT = L // 128  # 16 s-tiles / l-tiles
    NB = L // 512  # 4 OT banks

    with SplitWaitTileContext(nc) as tc:
        with (
            tc.tile_pool(name="const", bufs=1) as constp,
            tc.tile_pool(name="qk", bufs=2) as qkp,
            tc.tile_pool(name="vp", bufs=2) as vp,
            tc.tile_pool(name="ap", bufs=3) as ap_pool,
            tc.tile_pool(name="ep", bufs=2) as ep,
            tc.tile_pool(name="outp", bufs=2) as outp,
            tc.tile_pool(name="st", bufs=1, space="PSUM") as stp,
            tc.tile_pool(name="otp", bufs=2, space="PSUM") as otp,
            tc.tile_pool(name="ottp", bufs=2, space="PSUM") as ottp,
        ):
            mask_sb = constp.tile([128, 128], av_dtype, tag="mask")
            nc.sync.dma_start(out=mask_sb, in_=mask[:])
            ident_sb = constp.tile([128, 128], f32, tag="ident")
            nc.sync.dma_start(out=ident_sb, in_=ident[:])

            for pair in range(PAIRS_PER_CORE):
                # Q^T/K^T duplicated on partitions 0-63 and 64-127 so the
                # k=64 score matmuls can row-pack two s-tiles concurrently
                qt_sb = qkp.tile([2 * E, L], st_dtype, tag="qt")
                kt_sb = qkp.tile([2 * E, L], st_dtype, tag="kt")
                # V slab [128, 16, 66]; col 64 carries exp(delta') for the
                # softmax denominator (host-folded), col 65 is padding
                v_sb = vp.tile([128, NT, VW], av_dtype, tag="v")
                vv_r = vv[pair].rearrange("(t p) e -> p t e", p=128)
                # chunked loads (512 cols / 4 t-rows at a time) so the first
                # score group starts long before the full slabs land; the
                # duplicate partition-half loads ride the idle gpsimd (SWDGE)
                # queue
                for ch in range(4):
                    cl = slice(512 * ch, 512 * (ch + 1))
                    nc.sync.dma_start(out=kt_sb[0:E, cl], in_=kt[pair][:, cl])
                    nc.gpsimd.dma_start(
                        out=kt_sb[E : 2 * E, cl], in_=kt[pair][:, cl]
                    )
                    nc.sync.dma_start(out=qt_sb[0:E, cl], in_=qt[pair][:, cl])
                    nc.gpsimd.dma_start(
                        out=qt_sb[E : 2 * E, cl], in_=qt[pair][:, cl]
                    )
                    nc.sync.dma_start(
                        out=v_sb[:, 4 * ch : 4 * ch + 4, :],
                        in_=vv_r[:, 4 * ch : 4 * ch + 4, :],
                    )

                out_sb = outp.tile([128, NT, E], f32, tag="out")

                ot_banks = {}

                def emit_st_group(lj, gi):
                    a_grp = ap_pool.tile(
                        [128, 4 * 512], av_dtype, tag="A", name="A"
                    )
                    for hb in range(2):  # two double-buffered [128,1024] halves
                        st = stp.tile(
                            [128, 1024], f32, tag=f"st{hb}", name="st"
                        )
                        for cc in range(2):
                            c = 2 * hb + cc
                            si = 4 * gi + c
                            off = 128 * c if gi == lj else 0
                            half = (c % 2) * E
                            nc.tensor.matmul(
                                st[:, 512 * cc + off : 512 * (cc + 1)],
                                kt_sb[half : half + E, si * 128 : si * 128 + 128],
                                qt_sb[half : half + E, 512 * lj + off : 512 * lj + 512],
                                start=True,
                                stop=True,
                            )
                        if gi == lj:
                            # diagonal group: exp only the valid suffix of
                            # each 512-chunk (skips unwritten PSUM columns)
                            for cc in range(2):
                                c = 2 * hb + cc
                                off = 128 * c
                                nc.scalar.activation(
                                    out=a_grp[
                                        :, 512 * c + off : 512 * (c + 1)
                                    ],
                                    in_=st[:, 512 * cc + off : 512 * (cc + 1)],
                                    func=Exp,
                                    scale=1.0,
                                )
                        else:
                            nc.scalar.activation(
                                out=a_grp[:, 1024 * hb : 1024 * (hb + 1)],
                                in_=st,
                                func=Exp,
                                scale=1.0,
                            )
                    if gi == lj:
                        for c in range(4):
                            colb = 512 * c + 128 * c
                            nc.vector.tensor_mul(
                                a_grp[:, colb : colb + 128],
                                a_grp[:, colb : colb + 128],
                                mask_sb,
                            )
                    return a_grp

                def emit_av_group(lj, gi, a_grp):
                    ot = ot_banks[lj]
                    for c in range(4):
                        si = 4 * gi + c
                        off = 128 * c if gi == lj else 0
                        nc.tensor.matmul(
                            ot[:, off:512],
                            v_sb[:, si, 0 : E + 1],
                            a_grp[:, 512 * c + off : 512 * (c + 1)],
                            start=(gi == 0 and c == 0),
                            stop=(gi == lj and c == 3),
                        )

                def emit_epilogue(lj):
                    ot = ot_banks.pop(lj)
                    ot_sb = ep.tile([E + 1, 512], f32, tag="ot_sb", name="ot_sb")
                    nc.vector.tensor_copy(ot_sb, ot)
                    for c in range(4):
                        lt = 4 * lj + c
                        ott = ottp.tile([128, 512], f32, tag="ott", name="ott")
                        nc.tensor.transpose(
                            ott[:, 0 : E + 1],
                            ot_sb[:, c * 128 : (c + 1) * 128],
                            ident_sb[0 : E + 1, 0 : E + 1],
                        )
                        recip = ep.tile([128, 1], f32, tag="recip", name="recip")
                        nc.vector.reciprocal(recip, ott[:, E : E + 1])
                        nc.vector.tensor_scalar_mul(
                            out_sb[:, lt, :], ott[:, 0:E], recip
                        )

                # groups: (lj, gi) — bank lj accumulates s-tiles 0..4lj+3 in
                # groups of 4; gi == lj is the diagonal (partial) group.
                # Software-pipelined: PE stays one group ahead of AV.
                groups = [(lj, gi) for lj in range(NB) for gi in range(lj + 1)]
                prev = None
                for lj, gi in groups:
                    if lj not in ot_banks:
                        ot_banks[lj] = otp.tile(
                            [E + 1, 512], f32, tag="ot", name="ot"
                        )
                    a_grp = emit_st_group(lj, gi)
                    if prev is not None:
                        plj, pgi, pa = prev
                        emit_av_group(plj, pgi, pa)
                        if pgi == plj:  # that was the last group of bank plj
                            emit_epilogue(plj)
                    prev = (lj, gi, a_grp)
                plj, pgi, pa = prev
                emit_av_group(plj, pgi, pa)
                emit_epilogue(plj)

                nc.sync.dma_start(
                    out=oo[pair].rearrange("(t p) e -> p t e", p=128),
                    in_=out_sb,
                )

    return nc


# ---------------------------------------------------------------------------
# Host-side sharding / unsharding
# ---------------------------------------------------------------------------

def _in_maps(queries, keys, values, tau, delta, st_dtype=f32r, av_dtype=f32r):
    np_st = mybir.dt.np(st_dtype)
    np_av = mybir.dt.np(av_dtype)
    mask = np.triu(np.ones((128, 128), dtype=np.float32)).astype(np_av)
    ident = np.eye(128, dtype=np.float32)
    maps = []
    for c in range(N_CORES):
        ps = [2 * c, 2 * c + 1]
        b = ps[0] // H
        hs = [p % H for p in ps]
        qscale = np.float32(SCALE * tau[b, 0])
        qt = np.ascontiguousarray(
            np.stack([queries[b, :, h, :].T * qscale for h in hs])
        ).astype(np_st)
        kt = np.ascontiguousarray(
            np.stack([keys[b, :, h, :].T for h in hs])
        ).astype(np_st)
        # V augmented with the delta fold: cols 0..63 = V * exp(delta'),
        # col 64 = exp(delta') (denominator), col 65 pad
        expd = np.exp(SCALE * delta[b]).astype(np.float32)  # [L]
        vv = np.zeros((PAIRS_PER_CORE, L, E + 2), dtype=np.float32)
        for i, h in enumerate(hs):
            vv[i, :, 0:E] = values[b, :, h, :] * expd[:, None]
            vv[i, :, E] = expd
        vv = np.ascontiguousarray(vv).astype(np_av)
        maps.append(
            {"qt": qt, "kt": kt, "vv": vv, "mask": mask, "ident": ident}
        )
    return maps


_CACHED = {}


def run(queries, keys, values, tau, delta, trace=False, st_dtype=f32r,
        av_dtype=f32r):
    key = (str(st_dtype), str(av_dtype))
    if key not in _CACHED:
        _CACHED[key] = build_program(st_dtype, av_dtype)
    nc = _CACHED[key]
    in_maps = _in_maps(
        np.asarray(queries),
        np.asarray(keys),
        np.asarray(values),
        np.asarray(tau),
        np.asarray(delta),
        st_dtype=st_dtype,
        av_dtype=av_dtype,
    )
    res = run_bass_kernel_spmd(
        nc, in_maps, core_ids=list(range(N_CORES)), trace=trace
    )
    out = np.empty((B, L, H, E), dtype=np.float32)
    for c in range(N_CORES):
        o = res.results[c]["oo"]
        for i, p in enumerate([2 * c, 2 * c + 1]):
            out[p // H, :, p % H, :] = o[i]
    return out, res


def kernel(queries, keys, values, tau, delta):
    out, _ = run(queries, keys, values, tau, delta, trace=False)
    return out


# revision 18
# speedup vs baseline: 1.0659x; 1.0496x over previous
"""De-stationary causal attention (B=2, L=S=2048, H=8, E=64) on 8 TRN2 cores.

Sharding: the 16 (batch, head) pairs are distributed 2-per-core (cores 0-3
get batch 0, heads 0..7; cores 4-7 get batch 1). Each core runs the same
Bass program on its two pairs.

Math: logits = (Q K^T) * (tau/sqrt(E)) + delta/sqrt(E), causal softmax, A V.
Host-side folds: Q is pre-scaled by tau/sqrt(E); exp(delta/sqrt(E)) is folded
into V (and into the appended denominator column), because
softmax(x + d)_s = exp(x_s) e^{d_s} / sum_j exp(x_j) e^{d_j}.
So the device only computes exp(q'k) with no bias, letting one ACT call span
a whole 4-bank PSUM group.

Device structure per (b,h) pair, scores kept TRANSPOSED (s on partitions):
  bank-major over 4 output l-blocks of 512; for each bank, groups of 4
  s-tiles: ST[s,l] row-packed on the PE (two k=64 matmuls on partition halves
  run concurrently), one exp over the [128,2048] group, causal mask on diag
  blocks, then AV row-packed into two accumulators (k split 64+64), merged on
  the DVE, PE-transposed, normalized by the denominator column, stored.
"""

import copy
import sys

import numpy as np

try:
    import concourse.bass as bass
except ImportError:  # pragma: no cover
    sys.path.insert(0, "/opt/trn_rl_repo")
    import concourse.bass as bass

import concourse.mybir as mybir
import concourse.tile as tile
from concourse.bass_utils import run_bass_kernel_spmd
from concourse.vector_clock import ScopedClock

B, L, H, E = 2, 2048, 8, 64
N_CORES = 8
PAIRS_PER_CORE = 2
SCALE = 1.0 / np.sqrt(np.float32(E))  # 0.125

f32 = mybir.dt.float32
f32r = mybir.dt.float32r
bf16 = mybir.dt.bfloat16

# ---------------------------------------------------------------------------
# Walrus in this toolchain rejects >1 sync-wait per instruction. Split extra
# waits onto NoOps committed just before the instruction on the same engine.
# ---------------------------------------------------------------------------
_NOP_TEMPLATE = {}


def _make_nop(engine, name):
    if engine not in _NOP_TEMPLATE:
        tmp = bass.Bass()
        _NOP_TEMPLATE[engine] = tmp.engines[engine].nop(nofuse=True).ins
    nop = copy.copy(_NOP_TEMPLATE[engine])
    nop.name = name
    nop.engine = engine
    nop.sync_info = None
    return nop


class SplitWaitTileContext(tile.TileContext):
    _ws_counter = 0

    def _split_waits(self, inst):
        si = inst.sync_info
        if si is None or not si.on_wait or len(si.on_wait) <= 1:
            return []
        if inst.engine == mybir.EngineType.Unassigned:
            return []
        waits = list(si.on_wait)
        inst.sync_info = mybir.SyncInfo(
            on_wait=[waits[0]], on_update=list(si.on_update or [])
        )
        nops = []
        for w in waits[1:]:
            SplitWaitTileContext._ws_counter += 1
            nop = _make_nop(inst.engine, f"I-ws{SplitWaitTileContext._ws_counter}")
            nop.sync_info = mybir.SyncInfo(on_wait=[w], on_update=[])
            nops.append(nop)
        return nops

    def _commit_instruction(self, inst, lazy_reg_writes=True):
        for nop in self._split_waits(inst):
            self._add_instruction(nop)
        super()._commit_instruction(inst, lazy_reg_writes)

    def _drain_and_barrier(self, tick_clock, wait_clock):
        nc = self.nc
        probe = nc.sync.nop(nofuse=True)
        wait_clock.add_sem_waits(
            probe.ins, ScopedClock({None: tick_clock.global_clock})
        )
        waits = list(probe.ins.sync_info.on_wait or []) if probe.ins.sync_info else []
        if len(waits) > 1:
            probe.ins.sync_info.on_wait = [waits[0]]
            handles = {h.num: h for h in self.sems.allocated().values()}
            for w in waits[1:]:
                nop = nc.sync.nop(nofuse=True)
                nop.wait_op(handles[w.id], w.wait_value, "sem-ge")
        nc.sync.drain()

        nc.all_engine_barrier()
        assert self.sems is not None
        popped = nc._tile_sem_poison_stack.pop()
        assert popped is self._sem_poison
        nc.clear_and_free_semaphores(list(self.sems.allocated().values()))
        nc.all_engine_barrier()


# ---------------------------------------------------------------------------
# Program builder (bank-major, fully row-packed)
# ---------------------------------------------------------------------------

def build_program(st_dtype=f32r, av_dtype=f32r):
    nc = bass.Bass()
    Exp = mybir.ActivationFunctionType.Exp

    VW = E + 2  # v row: 64 values + denominator col + pad
    qt = nc.declare_dram_parameter("qt", [PAIRS_PER_CORE, E, L], st_dtype, isOutput=False)
    kt = nc.declare_dram_parameter("kt", [PAIRS_PER_CORE, E, L], st_dtype, isOutput=False)
    vv = nc.declare_dram_parameter("vv", [PAIRS_PER_CORE, L, VW], av_dtype, isOutput=False)
    mask = nc.declare_dram_parameter("mask", [128, 128], av_dtype, isOutput=False)
    ident = nc.declare_dram_parameter("ident", [128, 128], f32, isOutput=False)
    oo = nc.declare_dram_parameter("oo", [PAIRS_PER_CORE, L, E], f32, isOutput=True)

    NT = L // 128  # 16 s-tiles / l-tiles
    NB = L // 512  # 4 OT banks

    with SplitWaitTileContext(nc) as tc:
        with (
            tc.tile_pool(name="const", bufs=1) as constp,
            tc.tile_pool(name="qk", bufs=2) as qkp,
            tc.tile_pool(name="vp", bufs=2) as vp,
            tc.tile_pool(name="ap", bufs=3) as ap_pool,
            tc.tile_pool(name="ep", bufs=2) as ep,
            tc.tile_pool(name="outp", bufs=2) as outp,
            tc.tile_pool(name="st", bufs=1, space="PSUM") as stp,
            tc.tile_pool(name="otp", bufs=1, space="PSUM") as otp,
            tc.tile_pool(name="ottp", bufs=2, space="PSUM") as ottp,
        ):
            mask_sb = constp.tile([128, 128], av_dtype, tag="mask")
            nc.sync.dma_start(out=mask_sb, in_=mask[:])
            ident_sb = constp.tile([128, 128], f32, tag="ident")
            nc.sync.dma_start(out=ident_sb, in_=ident[:])

            for pair in range(PAIRS_PER_CORE):
                # Q^T/K^T duplicated on partitions 0-63 and 64-127 so the
                # k=64 score matmuls can row-pack two s-tiles concurrently
                qt_sb = qkp.tile([2 * E, L], st_dtype, tag="qt")
                kt_sb = qkp.tile([2 * E, L], st_dtype, tag="kt")
                # V slab [128, 16, 66]; col 64 carries exp(delta') for the
                # softmax denominator (host-folded), col 65 is padding
                v_sb = vp.tile([128, NT, VW], av_dtype, tag="v")
                vv_r = vv[pair].rearrange("(t p) e -> p t e", p=128)
                # chunked loads (512 cols / 4 t-rows at a time) so the first
                # score group starts long before the full slabs land; the
                # duplicate partition-half loads ride the idle gpsimd (SWDGE)
                # queue
                for ch in range(4):
                    cl = slice(512 * ch, 512 * (ch + 1))
                    nc.sync.dma_start(out=kt_sb[0:E, cl], in_=kt[pair][:, cl])
                    nc.gpsimd.dma_start(
                        out=kt_sb[E : 2 * E, cl], in_=kt[pair][:, cl]
                    )
                    nc.sync.dma_start(out=qt_sb[0:E, cl], in_=qt[pair][:, cl])
                    nc.gpsimd.dma_start(
                        out=qt_sb[E : 2 * E, cl], in_=qt[pair][:, cl]
                    )
                    nc.sync.dma_start(
                        out=v_sb[:, 4 * ch : 4 * ch + 4, :],
                        in_=vv_r[:, 4 * ch : 4 * ch + 4, :],
                    )

                out_sb = outp.tile([128, NT, E], f32, tag="out")

                ot_banks = {}

                def emit_st_group(lj, gi):
                    a_grp = ap_pool.tile(
                        [128, 4 * 512], av_dtype, tag="A", name="A"
                    )
                    for hb in range(2):  # two double-buffered [128,1024] halves
                        st = stp.tile(
                            [128, 1024], f32, tag=f"st{hb}", name="st"
                        )
                        for cc in range(2):
                            c = 2 * hb + cc
                            si = 4 * gi + c
                            off = 128 * c if gi == lj else 0
                            half = (c % 2) * E
                            nc.tensor.matmul(
                                st[:, 512 * cc + off : 512 * (cc + 1)],
                                kt_sb[half : half + E, si * 128 : si * 128 + 128],
                                qt_sb[half : half + E, 512 * lj + off : 512 * lj + 512],
                                start=True,
                                stop=True,
                            )
                        if gi == lj:
                            # diagonal group: exp only the valid suffix of
                            # each 512-chunk (skips unwritten PSUM columns)
                            for cc in range(2):
                                c = 2 * hb + cc
                                off = 128 * c
                                nc.scalar.activation(
                                    out=a_grp[
                                        :, 512 * c + off : 512 * (c + 1)
                                    ],
                                    in_=st[:, 512 * cc + off : 512 * (cc + 1)],
                                    func=Exp,
                                    scale=1.0,
                                )
                        else:
                            nc.scalar.activation(
                                out=a_grp[:, 1024 * hb : 1024 * (hb + 1)],
                                in_=st,
                                func=Exp,
                                scale=1.0,
                            )
                    if gi == lj:
                        for c in range(4):
                            colb = 512 * c + 128 * c
                            nc.vector.tensor_mul(
                                a_grp[:, colb : colb + 128],
                                a_grp[:, colb : colb + 128],
                                mask_sb,
                            )
                    return a_grp

                def emit_av_group(lj, gi, a_grp):
                    ot = ot_banks[lj]
                    for c in range(4):
                        si = 4 * gi + c
                        off = 128 * c if gi == lj else 0
                        nc.tensor.matmul(
                            ot[:, off:512],
                            v_sb[:, si, 0 : E + 1],
                            a_grp[:, 512 * c + off : 512 * (c + 1)],
                            start=(gi == 0 and c == 0),
                            stop=(gi == lj and c == 3),
                        )

                def emit_epilogue(lj):
                    ot = ot_banks.pop(lj)
                    ot_sb = ep.tile([E + 1, 512], f32, tag="ot_sb", name="ot_sb")
                    nc.vector.tensor_copy(ot_sb, ot)
                    for c in range(4):
                        lt = 4 * lj + c
                        ott = ottp.tile([128, 512], f32, tag="ott", name="ott")
                        nc.tensor.transpose(
                            ott[:, 0 : E + 1],
                            ot_sb[:, c * 128 : (c + 1) * 128],
                            ident_sb[0 : E + 1, 0 : E + 1],
                        )
                        recip = ep.tile([128, 1], f32, tag="recip", name="recip")
                        nc.vector.reciprocal(recip, ott[:, E : E + 1])
                        nc.vector.tensor_scalar_mul(
                            out_sb[:, lt, :], ott[:, 0:E], recip
                        )

                # groups: (lj, gi) — bank lj accumulates s-tiles 0..4lj+3 in
                # groups of 4; gi == lj is the diagonal (partial) group.
                # Software-pipelined: PE stays one group ahead of AV.
                groups = [(lj, gi) for lj in range(NB) for gi in range(lj + 1)]
                prev = None
                for lj, gi in groups:
                    if lj not in ot_banks:
                        ot_banks[lj] = otp.tile(
                            [E + 1, 512], f32, tag="ot", name="ot"
                        )
                    a_grp = emit_st_group(lj, gi)
                    if prev is not None:
                        plj, pgi, pa = prev
                        emit_av_group(plj, pgi, pa)
                        if pgi == plj:  # that was the last group of bank plj
                            emit_epilogue(plj)
                    prev = (lj, gi, a_grp)
                plj, pgi, pa = prev
                emit_av_group(plj, pgi, pa)
                emit_epilogue(plj)

                nc.sync.dma_start(
                    out=oo[pair].rearrange("(t p) e -> p t e", p=128),
                    in_=out_sb,
                )

    return nc


# ---------------------------------------------------------------------------
# Host-side sharding / unsharding
# ---------------------------------------------------------------------------

def _in_maps(queries, keys, values, tau, delta, st_dtype=f32r, av_dtype=f32r):
    np_st = mybir.dt.np(st_dtype)
    np_av = mybir.dt.np(av_dtype)
    mask = np.triu(np.ones((128, 128), dtype=np.float32)).astype(np_av)
    ident = np.eye(128, dtype=np.float32)
    maps = []
    for c in range(N_CORES):
        ps = [2 * c, 2 * c + 1]
        b = ps[0] // H
        hs = [p % H for p in ps]
        qscale = np.float32(SCALE * tau[b, 0])
        qt = np.ascontiguousarray(
            np.stack([queries[b, :, h, :].T * qscale for h in hs])
        ).astype(np_st)
        kt = np.ascontiguousarray(
            np.stack([keys[b, :, h, :].T for h in hs])
        ).astype(np_st)
        # V augmented with the delta fold: cols 0..63 = V * exp(delta'),
        # col 64 = exp(delta') (denominator), col 65 pad
        expd = np.exp(SCALE * delta[b]).astype(np.float32)  # [L]
        vv = np.zeros((PAIRS_PER_CORE, L, E + 2), dtype=np.float32)
        for i, h in enumerate(hs):
            vv[i, :, 0:E] = values[b, :, h, :] * expd[:, None]
            vv[i, :, E] = expd
        vv = np.ascontiguousarray(vv).astype(np_av)
        maps.append(
            {"qt": qt, "kt": kt, "vv": vv, "mask": mask, "ident": ident}
        )
    return maps


_CACHED = {}


def run(queries, keys, values, tau, delta, trace=False, st_dtype=f32r,
        av_dtype=f32r):
    key = (str(st_dtype), str(av_dtype))
    if key not in _CACHED:
        _CACHED[key] = build_program(st_dtype, av_dtype)
    nc = _CACHED[key]
    in_maps = _in_maps(
        np.asarray(queries),
        np.asarray(keys),
        np.asarray(values),
        np.asarray(tau),
        np.asarray(delta),
        st_dtype=st_dtype,
        av_dtype=av_dtype,
    )
    res = run_bass_kernel_spmd(
        nc, in_maps, core_ids=list(range(N_CORES)), trace=trace
    )
    out = np.empty((B, L, H, E), dtype=np.float32)
    for c in range(N_CORES):
        o = res.results[c]["oo"]
        for i, p in enumerate([2 * c, 2 * c + 1]):
            out[p // H, :, p % H, :] = o[i]
    return out, res


def kernel(queries, keys, values, tau, delta):
    out, _ = run(queries, keys, values, tau, delta, trace=False)
    return out
